# revision 15
# baseline (speedup 1.0000x reference)
"""MoE routing kernel for Trainium2 (8 NeuronCores, expert-parallel, fp8).

out[i] = x[i] + relu(x[i] @ W[e].T + b[e]),  e = cam_pred_ids[i]

Strategy: route tokens by expert on the host, so core e computes ONLY
expert e's tokens with ONLY W[e]. The matmul runs in fp8 e4m3 with
DoubleRow perf mode (two 128-deep K planes contracted per instruction,
2x the f16 PE rate), with W pre-scaled by 32 on the host to stay in
e4m3 normal range; the activation step rescales by 1/32 and applies
bias+relu. The residual is added from a separate f16 copy of x and the
output is stored as f16 (error is dominated by the fp8 weights;
measured end-to-end rel err ~1.4e-2, within the 2e-2 budget).

Layouts are pre-transposed on the host so every DMA moves >=2KB
contiguous per partition. Ring plan: sync = weight stream + output
stores (interleaved, emitted in-loop); gpsimd = x8 loads only (its
expensive dge_drain then fires early, hidden under the PE window);
scalar = bias + f16 residual + activations; vector = residual adds.
"""

import os
import numpy as np
import ml_dtypes

import concourse.bass as bass
from concourse import bacc
import concourse.mybir as mybir
import concourse.tile as tile
from concourse.bass_utils import run_bass_kernel_spmd

NUM_EXPERTS = 8
DIM = 2048
KT2 = DIM // 256  # 8 double-row k groups (256 contraction each)
OT = DIM // 128  # 16 o-tiles

W_SCALE = 32.0
MODE = "fp8dr"  # fp8 e4m3 DoubleRow
WPREFETCH = 4  # weight tiles requested ahead of the consuming o-tile


def _chunks(np_tokens: int) -> list[tuple[int, int]]:
    """Split the free dim into near-equal matmul chunks of <=512 (PSUM
    bank limit). Equal sizes keep every chunk's stream time above the
    ~140ns LDWEIGHTS time so weight reloads stay hidden."""
    n = (np_tokens + 511) // 512  # minimum chunk count
    out = []
    pos = 0
    for ci in range(n):
        take = (np_tokens - pos + (n - ci) - 1) // (n - ci)
        out.append((pos, take))
        pos += take
    return out


def _build_nc(np_tokens: int):
    f32 = mybir.dt.float32
    f16 = mybir.dt.float16
    f8 = mybir.dt.float8e4

    nc = bacc.Bacc()
    # wt[ot, k_lo, kt2, i, o] = 32*W[e][128*ot+o, 256*kt2+128*i+k_lo]
    wt_d = nc.declare_dram_parameter("wt", [OT, 128, KT2, 2, 128], f8, isOutput=False)
    # x8[k_lo, kt2, i, n] = fp8(x[n, 256*kt2+128*i+k_lo])
    x8_d = nc.declare_dram_parameter("x8", [128, KT2, 2, np_tokens], f8, isOutput=False)
    # xl[k_lo, kt2, i, n] = fp8(16*(x - dequant(x8)))  (residual lo plane)
    xl_d = nc.declare_dram_parameter("xl", [128, KT2, 2, np_tokens], f8, isOutput=False)
    b_d = nc.declare_dram_parameter("b", [128, OT], f32, isOutput=False)
    out_d = nc.declare_dram_parameter("out", [OT, 128, np_tokens], f16, isOutput=True)

    chunks = _chunks(np_tokens)
    relu = mybir.ActivationFunctionType.Relu
    dr = mybir.MatmulPerfMode.DoubleRow

    with tile.TileContext(nc) as tc:
        with (
            tc.tile_pool(name="xp", bufs=1) as xp,
            tc.tile_pool(name="wp", bufs=WPREFETCH + 1) as wp,
            tc.tile_pool(name="op", bufs=3) as op,
            tc.tile_pool(name="bp", bufs=1) as bp,
            tc.tile_pool(name="pp", bufs=2, space="PSUM") as pp,
        ):
            # First weight tile split in two so the first matmul group can
            # start as soon as the first half lands.
            wtiles = {}
            wtiles[0] = wp.tile([128, KT2, 2, 128], f8, name="wtile", tag="w")
            for q in range(2):
                nc.sync.dma_start(
                    out=wtiles[0][:, q * 4 : (q + 1) * 4, :, :],
                    in_=wt_d[0, :, q * 4 : (q + 1) * 4, :, :],
                )

            btile = bp.tile([128, OT], f32, name="btile")
            nc.scalar.dma_start(out=btile, in_=b_d[:, :])

            # x8 split across three rings, slab arrival matched to the PE's
            # kt2 consumption order: gpsimd carries kt2 0-2 as singles,
            # sync slots kt2 3 between w0 and w1, scalar takes kt2 4-7.
            x8all = xp.tile([128, KT2, 2, np_tokens], f8, name="x8all")
            for s in (0, 1, 2):
                nc.gpsimd.dma_start(
                    out=x8all[:, s : s + 1, :, :],
                    in_=x8_d[:, s : s + 1, :, :],
                )
            nc.sync.dma_start(
                out=x8all[:, 3:4, :, :],
                in_=x8_d[:, 3:4, :, :],
            )
            for s in (4, 6):
                nc.scalar.dma_start(
                    out=x8all[:, s : s + 2, :, :],
                    in_=x8_d[:, s : s + 2, :, :],
                )

            # residual lo plane on the scalar ring (2 slabs); needed only at
            # the post-activation adds, so it queues behind the x8 slabs.
            # The residual is reconstructed on-chip as x8 + xl/16 — the f16
            # x copy is never loaded, saving 2.1MB of HBM traffic per core.
            xlall = xp.tile([128, KT2, 2, np_tokens], f8, name="xlall")
            for s in (0, 4):
                nc.scalar.dma_start(
                    out=xlall[:, s : s + 4, :, :],
                    in_=xl_d[:, s : s + 4, :, :],
                )

            # Prefetch the next few weight tiles; the rest are emitted
            # inside the o-tile loop so the sync ring interleaves them
            # with the output stores (FIFO per ring).
            for ot in range(1, WPREFETCH):
                wtiles[ot] = wp.tile([128, KT2, 2, 128], f8, name="wtile", tag="w")
                nc.sync.dma_start(out=wtiles[ot], in_=wt_d[ot])

            for ot in range(OT):
                wtile = wtiles[ot]
                otile = op.tile([128, np_tokens], f16, name="otile", tag="o")
                psums = [
                    pp.tile([128, ch], f32, name=f"ps{ci}", tag=f"ps{ci}")
                    for ci, (_, ch) in enumerate(chunks)
                ]
                last = ot == OT - 1
                if last:
                    # chunk-outer so chunk 0's act/add/store overlap the
                    # remaining chunks' matmuls (shrinks the tail)
                    for ci, (n0, ch) in enumerate(chunks):
                        for kt2 in range(KT2):
                            nc.tensor.matmul(
                                psums[ci],
                                wtile[:, kt2, :, :],
                                x8all[:, kt2, :, n0 : n0 + ch],
                                start=(kt2 == 0),
                                stop=(kt2 == KT2 - 1),
                                perf_mode=dr,
                            )
                else:
                    for kt2 in range(KT2):
                        lhsT = wtile[:, kt2, :, :]
                        for ci, (n0, ch) in enumerate(chunks):
                            nc.tensor.matmul(
                                psums[ci],
                                lhsT,
                                x8all[:, kt2, :, n0 : n0 + ch],
                                start=(kt2 == 0),
                                stop=(kt2 == KT2 - 1),
                                perf_mode=dr,
                            )
                # residual views: d = 128*ot + p lives at kt2=ot//2, i=ot%2
                xhi = x8all[:, ot // 2, ot % 2, :]
                xlo = xlall[:, ot // 2, ot % 2, :]
                for ci, (n0, ch) in enumerate(chunks):
                    nc.scalar.activation(
                        otile[:, n0 : n0 + ch],
                        psums[ci],
                        relu,
                        bias=btile[:, ot : ot + 1],
                        scale=1.0 / W_SCALE,
                    )
                    nc.gpsimd.tensor_add(
                        otile[:, n0 : n0 + ch],
                        otile[:, n0 : n0 + ch],
                        xhi[:, n0 : n0 + ch],
                    )
                    nc.vector.scalar_tensor_tensor(
                        otile[:, n0 : n0 + ch],
                        xlo[:, n0 : n0 + ch],
                        1.0 / 16.0,
                        otile[:, n0 : n0 + ch],
                        mybir.AluOpType.mult,
                        mybir.AluOpType.add,
                    )
                    if last:
                        # per-chunk stores on alternating rings to
                        # parallelize the tail
                        eng = nc.scalar if ci == 1 else nc.sync
                        eng.dma_start(
                            out=out_d[ot, :, n0 : n0 + ch],
                            in_=otile[:, n0 : n0 + ch],
                        )
                if not last:
                    nc.sync.dma_start(out=out_d[ot], in_=otile)
                nxt = ot + WPREFETCH
                if nxt < OT:
                    wtiles[nxt] = wp.tile(
                        [128, KT2, 2, 128], f8, name="wtile", tag="w"
                    )
                    nc.sync.dma_start(out=wtiles[nxt], in_=wt_d[nxt])
    nc.compile()
    return nc


def kernel(x, cam_pred_ids, W, b, _want_results=False):
    x = np.ascontiguousarray(np.asarray(x), dtype=np.float32)
    W = np.asarray(W, dtype=np.float32)
    b = np.asarray(b, dtype=np.float32)
    ids = np.asarray(cam_pred_ids).astype(np.int64)
    f8 = ml_dtypes.float8_e4m3

    counts = np.bincount(ids, minlength=NUM_EXPERTS)
    order = np.argsort(ids, kind="stable")
    np_tokens = max(512, int(counts.max()))

    # per-expert padded token index lists (pad with token 0; discarded later)
    starts = np.zeros(NUM_EXPERTS + 1, dtype=np.int64)
    np.cumsum(counts, out=starts[1:])
    idx = np.zeros((NUM_EXPERTS, np_tokens), dtype=np.int64)
    for e in range(NUM_EXPERTS):
        idx[e, : counts[e]] = order[starts[e] : starts[e + 1]]

    in_maps = []
    for e in range(NUM_EXPERTS):
        xg = x[idx[e]]  # [Np, DIM]
        xt = np.ascontiguousarray(xg.T)  # [DIM, Np]
        # x8[k_lo, kt2, i, n] plus the fp8 lo plane of the residual split
        xtl = np.ascontiguousarray(
            xt.reshape(KT2, 2, 128, np_tokens).transpose(2, 0, 1, 3)
        )
        x8 = xtl.astype(f8)
        xl = ((xtl - x8.astype(np.float32)) * 16.0).astype(f8)
        # wt[ot, k_lo, kt2, i, o] = 32*W[e][128*ot+o, 256*kt2+128*i+k_lo]
        wt = np.ascontiguousarray(
            (W_SCALE * W[e])
            .reshape(OT, 128, KT2, 2, 128)
            .transpose(0, 4, 2, 3, 1),
            dtype=f8,
        )
        m = {
            "wt": wt,
            "x8": x8,
            "xl": xl,
            "b": np.ascontiguousarray(b[e].reshape(OT, 128).T),
        }
        in_maps.append(m)

    nc = _build_nc(np_tokens)
    res = run_bass_kernel_spmd(
        nc,
        in_maps,
        core_ids=list(range(NUM_EXPERTS)),
        trace=bool(int(os.environ.get("BASS_MOE_TRACE", "0"))),
    )

    out = np.empty_like(x)
    for e in range(NUM_EXPERTS):
        oute = res.results[e]["out"]  # [OT, 128, Np] f16
        valid = idx[e, : counts[e]]
        out[valid] = (
            oute.reshape(DIM, np_tokens).T[: counts[e]].astype(np.float32)
        )
    if _want_results:
        return out, res
    return out


# revision 18
# speedup vs baseline: 1.0196x; 1.0196x over previous
"""MoE routing kernel for Trainium2 (8 NeuronCores, expert-parallel, fp8).

out[i] = x[i] + relu(x[i] @ W[e].T + b[e]),  e = cam_pred_ids[i]

Strategy: route tokens by expert on the host, so core e computes ONLY
expert e's tokens with ONLY W[e]. The matmul runs in fp8 e4m3 with
DoubleRow perf mode (two 128-deep K planes contracted per instruction,
2x the f16 PE rate), with W pre-scaled by 32 on the host to stay in
e4m3 normal range; the activation step rescales by 1/32 and applies
bias+relu. The residual is added from a separate f16 copy of x and the
output is stored as f16 (error is dominated by the fp8 weights;
measured end-to-end rel err ~1.4e-2, within the 2e-2 budget).

Layouts are pre-transposed on the host so every DMA moves >=2KB
contiguous per partition. Ring plan: sync = weight stream + output
stores (interleaved, emitted in-loop); gpsimd = x8 loads only (its
expensive dge_drain then fires early, hidden under the PE window);
scalar = bias + f16 residual + activations; vector = residual adds.
"""

import os
import numpy as np
import ml_dtypes

import concourse.bass as bass
from concourse import bacc
import concourse.mybir as mybir
import concourse.tile as tile
from concourse.bass_utils import run_bass_kernel_spmd

NUM_EXPERTS = 8
DIM = 2048
KT2 = DIM // 256  # 8 double-row k groups (256 contraction each)
OT = DIM // 128  # 16 o-tiles

W_SCALE = 32.0
MODE = "fp8dr"  # fp8 e4m3 DoubleRow
WPREFETCH = 5  # weight tiles requested ahead of the consuming o-tile


def _chunks(np_tokens: int) -> list[tuple[int, int]]:
    """Split the free dim into near-equal matmul chunks of <=512 (PSUM
    bank limit). Equal sizes keep every chunk's stream time above the
    ~140ns LDWEIGHTS time so weight reloads stay hidden."""
    n = (np_tokens + 511) // 512  # minimum chunk count
    out = []
    pos = 0
    for ci in range(n):
        take = (np_tokens - pos + (n - ci) - 1) // (n - ci)
        out.append((pos, take))
        pos += take
    return out


def _build_nc(np_tokens: int):
    f32 = mybir.dt.float32
    f16 = mybir.dt.float16
    f8 = mybir.dt.float8e4

    nc = bacc.Bacc()
    # wt[ot, k_lo, kt2, i, o] = 32*W[e][128*ot+o, 256*kt2+128*i+k_lo]
    wt_d = nc.declare_dram_parameter("wt", [OT, 128, KT2, 2, 128], f8, isOutput=False)
    # x8[k_lo, kt2, i, n] = fp8(x[n, 256*kt2+128*i+k_lo])
    x8_d = nc.declare_dram_parameter("x8", [128, KT2, 2, np_tokens], f8, isOutput=False)
    # xr[(dt p), n] = f16(x[n, 128*dt+p])   (residual, d on partitions)
    xr_d = nc.declare_dram_parameter("xr", [DIM, np_tokens], f16, isOutput=False)
    b_d = nc.declare_dram_parameter("b", [128, OT], f32, isOutput=False)
    out_d = nc.declare_dram_parameter("out", [OT, 128, np_tokens], f16, isOutput=True)

    chunks = _chunks(np_tokens)
    relu = mybir.ActivationFunctionType.Relu
    dr = mybir.MatmulPerfMode.DoubleRow

    with tile.TileContext(nc) as tc:
        with (
            tc.tile_pool(name="xp", bufs=1) as xp,
            tc.tile_pool(name="wp", bufs=WPREFETCH + 1) as wp,
            tc.tile_pool(name="op", bufs=3) as op,
            tc.tile_pool(name="bp", bufs=1) as bp,
            tc.tile_pool(name="pp", bufs=2, space="PSUM") as pp,
        ):
            # First weight tile split in four so the first matmul groups can
            # start as soon as the leading pieces land; x8 kt2=3 rides the
            # sync ring between them, matched to consumption order.
            wtiles = {}
            wtiles[0] = wp.tile([128, KT2, 2, 128], f8, name="wtile", tag="w")
            x8all = xp.tile([128, KT2, 2, np_tokens], f8, name="x8all")
            for q in range(2):
                nc.sync.dma_start(
                    out=wtiles[0][:, q * 2 : (q + 1) * 2, :, :],
                    in_=wt_d[0, :, q * 2 : (q + 1) * 2, :, :],
                )
            nc.sync.dma_start(
                out=x8all[:, 3:4, :, :],
                in_=x8_d[:, 3:4, :, :],
            )
            for q in range(2, 4):
                nc.sync.dma_start(
                    out=wtiles[0][:, q * 2 : (q + 1) * 2, :, :],
                    in_=wt_d[0, :, q * 2 : (q + 1) * 2, :, :],
                )

            btile = bp.tile([128, OT], f32, name="btile")
            nc.scalar.dma_start(out=btile, in_=b_d[:, :])

            # rest of x8: gpsimd carries kt2 0-2 singles (consumed first)
            # then kt2 7; scalar takes kt2 4-6.
            for s in (0, 1, 2, 7):
                nc.gpsimd.dma_start(
                    out=x8all[:, s : s + 1, :, :],
                    in_=x8_d[:, s : s + 1, :, :],
                )
            for s in (4, 6):
                w = 2 if s == 4 else 1
                nc.scalar.dma_start(
                    out=x8all[:, s : s + w, :, :],
                    in_=x8_d[:, s : s + w, :, :],
                )

            # f16 residual on the scalar ring in 4 slabs; slab dt covers
            # o-tiles 4dt..4dt+3, needed only at the post-activation adds,
            # so it queues behind the x8 slabs.
            xr_r = xr_d.rearrange("(t p) n -> p t n", p=128)
            xrall = xp.tile([128, OT, np_tokens], f16, name="xrall")
            for s in range(0, OT, 4):
                nc.scalar.dma_start(
                    out=xrall[:, s : s + 4, :],
                    in_=xr_r[:, s : s + 4, :],
                )

            # Prefetch the next few weight tiles; the rest are emitted
            # inside the o-tile loop so the sync ring interleaves them
            # with the output stores (FIFO per ring).
            for ot in range(1, WPREFETCH):
                wtiles[ot] = wp.tile([128, KT2, 2, 128], f8, name="wtile", tag="w")
                nc.sync.dma_start(out=wtiles[ot], in_=wt_d[ot])

            for ot in range(OT):
                wtile = wtiles[ot]
                otile = op.tile([128, np_tokens], f16, name="otile", tag="o")
                psums = [
                    pp.tile([128, ch], f32, name=f"ps{ci}", tag=f"ps{ci}")
                    for ci, (_, ch) in enumerate(chunks)
                ]
                last = ot == OT - 1
                if last:
                    # chunk-outer so chunk 0's act/add/store overlap the
                    # remaining chunks' matmuls (shrinks the tail)
                    for ci, (n0, ch) in enumerate(chunks):
                        for kt2 in range(KT2):
                            nc.tensor.matmul(
                                psums[ci],
                                wtile[:, kt2, :, :],
                                x8all[:, kt2, :, n0 : n0 + ch],
                                start=(kt2 == 0),
                                stop=(kt2 == KT2 - 1),
                                perf_mode=dr,
                            )
                else:
                    for kt2 in range(KT2):
                        lhsT = wtile[:, kt2, :, :]
                        for ci, (n0, ch) in enumerate(chunks):
                            nc.tensor.matmul(
                                psums[ci],
                                lhsT,
                                x8all[:, kt2, :, n0 : n0 + ch],
                                start=(kt2 == 0),
                                stop=(kt2 == KT2 - 1),
                                perf_mode=dr,
                            )
                for ci, (n0, ch) in enumerate(chunks):
                    nc.scalar.activation(
                        otile[:, n0 : n0 + ch],
                        psums[ci],
                        relu,
                        bias=btile[:, ot : ot + 1],
                        scale=1.0 / W_SCALE,
                    )
                    nc.vector.tensor_add(
                        otile[:, n0 : n0 + ch],
                        otile[:, n0 : n0 + ch],
                        xrall[:, ot, n0 : n0 + ch],
                    )
                    if last:
                        # per-chunk stores on alternating rings to
                        # parallelize the tail
                        eng = nc.scalar if ci == 1 else nc.sync
                        eng.dma_start(
                            out=out_d[ot, :, n0 : n0 + ch],
                            in_=otile[:, n0 : n0 + ch],
                        )
                if not last:
                    nc.sync.dma_start(out=out_d[ot], in_=otile)
                nxt = ot + WPREFETCH
                if nxt < OT:
                    wtiles[nxt] = wp.tile(
                        [128, KT2, 2, 128], f8, name="wtile", tag="w"
                    )
                    nc.sync.dma_start(out=wtiles[nxt], in_=wt_d[nxt])
    nc.compile()
    return nc


def kernel(x, cam_pred_ids, W, b, _want_results=False):
    x = np.ascontiguousarray(np.asarray(x), dtype=np.float32)
    W = np.asarray(W, dtype=np.float32)
    b = np.asarray(b, dtype=np.float32)
    ids = np.asarray(cam_pred_ids).astype(np.int64)
    f8 = ml_dtypes.float8_e4m3

    counts = np.bincount(ids, minlength=NUM_EXPERTS)
    order = np.argsort(ids, kind="stable")
    np_tokens = max(512, int(counts.max()))

    # per-expert padded token index lists (pad with token 0; discarded later)
    starts = np.zeros(NUM_EXPERTS + 1, dtype=np.int64)
    np.cumsum(counts, out=starts[1:])
    idx = np.zeros((NUM_EXPERTS, np_tokens), dtype=np.int64)
    for e in range(NUM_EXPERTS):
        idx[e, : counts[e]] = order[starts[e] : starts[e + 1]]

    in_maps = []
    for e in range(NUM_EXPERTS):
        xg = x[idx[e]]  # [Np, DIM]
        xt = np.ascontiguousarray(xg.T)  # [DIM, Np]
        # x8[k_lo, kt2, i, n]
        x8 = np.ascontiguousarray(
            xt.reshape(KT2, 2, 128, np_tokens).transpose(2, 0, 1, 3), dtype=f8
        )
        # wt[ot, k_lo, kt2, i, o] = 32*W[e][128*ot+o, 256*kt2+128*i+k_lo]
        wt = np.ascontiguousarray(
            (W_SCALE * W[e])
            .reshape(OT, 128, KT2, 2, 128)
            .transpose(0, 4, 2, 3, 1),
            dtype=f8,
        )
        m = {
            "wt": wt,
            "x8": x8,
            "xr": np.ascontiguousarray(xt, dtype=np.float16),
            "b": np.ascontiguousarray(b[e].reshape(OT, 128).T),
        }
        in_maps.append(m)

    nc = _build_nc(np_tokens)
    res = run_bass_kernel_spmd(
        nc,
        in_maps,
        core_ids=list(range(NUM_EXPERTS)),
        trace=bool(int(os.environ.get("BASS_MOE_TRACE", "0"))),
    )

    out = np.empty_like(x)
    for e in range(NUM_EXPERTS):
        oute = res.results[e]["out"]  # [OT, 128, Np] f16
        valid = idx[e, : counts[e]]
        out[valid] = (
            oute.reshape(DIM, np_tokens).T[: counts[e]].astype(np.float32)
        )
    if _want_results:
        return out, res
    return out


# revision 21
# speedup vs baseline: 1.0205x; 1.0009x over previous
"""MoE routing kernel for Trainium2 (8 NeuronCores, expert-parallel, fp8).

out[i] = x[i] + relu(x[i] @ W[e].T + b[e]),  e = cam_pred_ids[i]

Strategy: route tokens by expert on the host, so core e computes ONLY
expert e's tokens with ONLY W[e]. The matmul runs in fp8 e4m3 with
DoubleRow perf mode (two 128-deep K planes contracted per instruction,
2x the f16 PE rate), with W pre-scaled by 32 on the host to stay in
e4m3 normal range; the activation step rescales by 1/32 and applies
bias+relu. The residual is added from a separate f16 copy of x and the
output is stored as f16 (error is dominated by the fp8 weights;
measured end-to-end rel err ~1.4e-2, within the 2e-2 budget).

Layouts are pre-transposed on the host so every DMA moves >=2KB
contiguous per partition. Ring plan: sync = weight stream + output
stores (interleaved, emitted in-loop); gpsimd = x8 loads only (its
expensive dge_drain then fires early, hidden under the PE window);
scalar = bias + f16 residual + activations; vector = residual adds.
"""

import os
import numpy as np
import ml_dtypes

import concourse.bass as bass
from concourse import bacc
import concourse.mybir as mybir
import concourse.tile as tile
from concourse.bass_utils import run_bass_kernel_spmd

NUM_EXPERTS = 8
DIM = 2048
KT2 = DIM // 256  # 8 double-row k groups (256 contraction each)
OT = DIM // 128  # 16 o-tiles

W_SCALE = 32.0
MODE = "fp8dr"  # fp8 e4m3 DoubleRow
WPREFETCH = 5  # weight tiles requested ahead of the consuming o-tile
NDUMMY = 12  # PE-clock warmup matmuls before the real stream
# o-tile 0 consumes kt2 slabs in their DMA arrival order (slab 0-2 on
# gpsimd, 3 on sync, 4-6 on scalar, 7 on gpsimd) to avoid PE idle gaps,
# each of which would reset the clock ramp.
OT0_ORDER = (0, 3, 4, 1, 5, 2, 6, 7)


def _chunks(np_tokens: int) -> list[tuple[int, int]]:
    """Split the free dim into near-equal matmul chunks of <=512 (PSUM
    bank limit). Equal sizes keep every chunk's stream time above the
    ~140ns LDWEIGHTS time so weight reloads stay hidden."""
    n = (np_tokens + 511) // 512  # minimum chunk count
    out = []
    pos = 0
    for ci in range(n):
        take = (np_tokens - pos + (n - ci) - 1) // (n - ci)
        out.append((pos, take))
        pos += take
    return out


def _build_nc(np_tokens: int):
    f32 = mybir.dt.float32
    f16 = mybir.dt.float16
    f8 = mybir.dt.float8e4

    nc = bacc.Bacc()
    # wt[ot, k_lo, kt2, i, o] = 32*W[e][128*ot+o, 256*kt2+128*i+k_lo]
    wt_d = nc.declare_dram_parameter("wt", [OT, 128, KT2, 2, 128], f8, isOutput=False)
    # x8[k_lo, kt2, i, n] = fp8(x[n, 256*kt2+128*i+k_lo])
    x8_d = nc.declare_dram_parameter("x8", [128, KT2, 2, np_tokens], f8, isOutput=False)
    # xr[(dt p), n] = f16(x[n, 128*dt+p])   (residual, d on partitions)
    xr_d = nc.declare_dram_parameter("xr", [DIM, np_tokens], f16, isOutput=False)
    b_d = nc.declare_dram_parameter("b", [128, OT], f32, isOutput=False)
    out_d = nc.declare_dram_parameter("out", [OT, 128, np_tokens], f16, isOutput=True)

    chunks = _chunks(np_tokens)
    relu = mybir.ActivationFunctionType.Relu
    dr = mybir.MatmulPerfMode.DoubleRow

    with tile.TileContext(nc) as tc:
        with (
            tc.tile_pool(name="xp", bufs=1) as xp,
            tc.tile_pool(name="wp", bufs=WPREFETCH + 1) as wp,
            tc.tile_pool(name="op", bufs=3) as op,
            tc.tile_pool(name="bp", bufs=1) as bp,
            tc.tile_pool(name="pp", bufs=2, space="PSUM") as pp,
            tc.tile_pool(name="dp", bufs=1, space="PSUM") as dp,
        ):
            # Warmup: the PE clock ramps from 1.2GHz to 2.4GHz only after
            # ~3us of continuous execution, and any idle gap resets it.
            # Run dummy matmuls on scratch data while the first weight/x
            # DMAs are in flight so the real stream starts at full clock.
            wscr = bp.tile([128, 2, 128], f8, name="wscr")
            xscr = bp.tile([128, 2, 256], f8, name="xscr")
            pscr = dp.tile([128, 256], f32, name="pscr")
            nc.vector.memset(wscr, 0)
            nc.vector.memset(xscr, 0)
            for _ in range(NDUMMY):
                nc.tensor.matmul(
                    pscr, wscr, xscr, start=True, stop=True, perf_mode=dr
                )
            # First weight tile split in four so the first matmul groups can
            # start as soon as the leading pieces land; x8 kt2=3 rides the
            # sync ring between them, matched to consumption order.
            wtiles = {}
            wtiles[0] = wp.tile([128, KT2, 2, 128], f8, name="wtile", tag="w")
            x8all = xp.tile([128, KT2, 2, np_tokens], f8, name="x8all")
            for q in range(2):
                nc.sync.dma_start(
                    out=wtiles[0][:, q * 2 : (q + 1) * 2, :, :],
                    in_=wt_d[0, :, q * 2 : (q + 1) * 2, :, :],
                )
            nc.sync.dma_start(
                out=x8all[:, 3:4, :, :],
                in_=x8_d[:, 3:4, :, :],
            )
            for q in range(2, 4):
                nc.sync.dma_start(
                    out=wtiles[0][:, q * 2 : (q + 1) * 2, :, :],
                    in_=wt_d[0, :, q * 2 : (q + 1) * 2, :, :],
                )

            btile = bp.tile([128, OT], f32, name="btile")
            nc.scalar.dma_start(out=btile, in_=b_d[:, :])

            # rest of x8: gpsimd carries kt2 0-2 singles (consumed first)
            # then kt2 7; scalar takes kt2 4-6.
            for s in (0, 1, 2, 7):
                nc.gpsimd.dma_start(
                    out=x8all[:, s : s + 1, :, :],
                    in_=x8_d[:, s : s + 1, :, :],
                )
            for s in (4, 6):
                w = 2 if s == 4 else 1
                nc.scalar.dma_start(
                    out=x8all[:, s : s + w, :, :],
                    in_=x8_d[:, s : s + w, :, :],
                )

            # f16 residual on the scalar ring in 4 slabs; slab dt covers
            # o-tiles 4dt..4dt+3, needed only at the post-activation adds,
            # so it queues behind the x8 slabs.
            xr_r = xr_d.rearrange("(t p) n -> p t n", p=128)
            xrall = xp.tile([128, OT, np_tokens], f16, name="xrall")
            for s in range(0, OT, 4):
                nc.scalar.dma_start(
                    out=xrall[:, s : s + 4, :],
                    in_=xr_r[:, s : s + 4, :],
                )

            # Prefetch the next few weight tiles; the rest are emitted
            # inside the o-tile loop so the sync ring interleaves them
            # with the output stores (FIFO per ring).
            for ot in range(1, WPREFETCH):
                wtiles[ot] = wp.tile([128, KT2, 2, 128], f8, name="wtile", tag="w")
                nc.sync.dma_start(out=wtiles[ot], in_=wt_d[ot])

            for ot in range(OT):
                wtile = wtiles[ot]
                otile = op.tile([128, np_tokens], f16, name="otile", tag="o")
                psums = [
                    pp.tile([128, ch], f32, name=f"ps{ci}", tag=f"ps{ci}")
                    for ci, (_, ch) in enumerate(chunks)
                ]
                last = ot == OT - 1
                if last:
                    # chunk-outer so chunk 0's act/add/store overlap the
                    # remaining chunks' matmuls (shrinks the tail)
                    for ci, (n0, ch) in enumerate(chunks):
                        for kt2 in range(KT2):
                            nc.tensor.matmul(
                                psums[ci],
                                wtile[:, kt2, :, :],
                                x8all[:, kt2, :, n0 : n0 + ch],
                                start=(kt2 == 0),
                                stop=(kt2 == KT2 - 1),
                                perf_mode=dr,
                            )
                else:
                    kt2_order = OT0_ORDER if ot == 0 else range(KT2)
                    for j, kt2 in enumerate(kt2_order):
                        lhsT = wtile[:, kt2, :, :]
                        for ci, (n0, ch) in enumerate(chunks):
                            nc.tensor.matmul(
                                psums[ci],
                                lhsT,
                                x8all[:, kt2, :, n0 : n0 + ch],
                                start=(j == 0),
                                stop=(j == KT2 - 1),
                                perf_mode=dr,
                            )
                for ci, (n0, ch) in enumerate(chunks):
                    nc.scalar.activation(
                        otile[:, n0 : n0 + ch],
                        psums[ci],
                        relu,
                        bias=btile[:, ot : ot + 1],
                        scale=1.0 / W_SCALE,
                    )
                    nc.vector.tensor_add(
                        otile[:, n0 : n0 + ch],
                        otile[:, n0 : n0 + ch],
                        xrall[:, ot, n0 : n0 + ch],
                    )
                    if last:
                        # per-chunk stores on alternating rings to
                        # parallelize the tail
                        eng = nc.scalar if ci == 1 else nc.sync
                        eng.dma_start(
                            out=out_d[ot, :, n0 : n0 + ch],
                            in_=otile[:, n0 : n0 + ch],
                        )
                if not last:
                    nc.sync.dma_start(out=out_d[ot], in_=otile)
                nxt = ot + WPREFETCH
                if nxt < OT:
                    wtiles[nxt] = wp.tile(
                        [128, KT2, 2, 128], f8, name="wtile", tag="w"
                    )
                    nc.sync.dma_start(out=wtiles[nxt], in_=wt_d[nxt])
    nc.compile()
    return nc


def kernel(x, cam_pred_ids, W, b, _want_results=False):
    x = np.ascontiguousarray(np.asarray(x), dtype=np.float32)
    W = np.asarray(W, dtype=np.float32)
    b = np.asarray(b, dtype=np.float32)
    ids = np.asarray(cam_pred_ids).astype(np.int64)
    f8 = ml_dtypes.float8_e4m3

    counts = np.bincount(ids, minlength=NUM_EXPERTS)
    order = np.argsort(ids, kind="stable")
    np_tokens = max(512, int(counts.max()))

    # per-expert padded token index lists (pad with token 0; discarded later)
    starts = np.zeros(NUM_EXPERTS + 1, dtype=np.int64)
    np.cumsum(counts, out=starts[1:])
    idx = np.zeros((NUM_EXPERTS, np_tokens), dtype=np.int64)
    for e in range(NUM_EXPERTS):
        idx[e, : counts[e]] = order[starts[e] : starts[e + 1]]

    in_maps = []
    for e in range(NUM_EXPERTS):
        xg = x[idx[e]]  # [Np, DIM]
        xt = np.ascontiguousarray(xg.T)  # [DIM, Np]
        # x8[k_lo, kt2, i, n]
        x8 = np.ascontiguousarray(
            xt.reshape(KT2, 2, 128, np_tokens).transpose(2, 0, 1, 3), dtype=f8
        )
        # wt[ot, k_lo, kt2, i, o] = 32*W[e][128*ot+o, 256*kt2+128*i+k_lo]
        wt = np.ascontiguousarray(
            (W_SCALE * W[e])
            .reshape(OT, 128, KT2, 2, 128)
            .transpose(0, 4, 2, 3, 1),
            dtype=f8,
        )
        m = {
            "wt": wt,
            "x8": x8,
            "xr": np.ascontiguousarray(xt, dtype=np.float16),
            "b": np.ascontiguousarray(b[e].reshape(OT, 128).T),
        }
        in_maps.append(m)

    nc = _build_nc(np_tokens)
    res = run_bass_kernel_spmd(
        nc,
        in_maps,
        core_ids=list(range(NUM_EXPERTS)),
        trace=bool(int(os.environ.get("BASS_MOE_TRACE", "0"))),
    )

    out = np.empty_like(x)
    for e in range(NUM_EXPERTS):
        oute = res.results[e]["out"]  # [OT, 128, Np] f16
        valid = idx[e, : counts[e]]
        out[valid] = (
            oute.reshape(DIM, np_tokens).T[: counts[e]].astype(np.float32)
        )
    if _want_results:
        return out, res
    return out
